# revision 77
# baseline (speedup 1.0000x reference)
"""Trainium2 Bass kernel for nn_Attention_v4 (sparse per-atom attention).

Reference computation (fp32):
    x:[2,512,14,1024] -> qkv = x@w_qkv+b_qkv -> per (b, r=atom, head)
    attention over the n=512 axis -> out @ w_proj + b_proj.

Sharding (8 cores): 4 groups x 7 (b,r)-units data-parallel, x 2 head-halves
tensor-parallel. Each core computes, for its 7 units and its 8 heads:
QKV^T projection, attention, and a partial c_proj (contraction over its 512
of the 1024 hd rows). Host unshard sums the two head-half partials and adds
b_proj.

Schedule (all engine/latency choices HW-measured on these cores; 603us
baseline -> ~406-417us through the changes below):
- unit-level software pipeline: stage s emits dma_x(s+2) then interleaves
  [ attn(s) : proj(s+1) + cproj(s-1) ] filler matmul GROUPS spread evenly
  over the 48 attention yields (3 head-pairs of exp->PAV lookahead).
- the two K=64 score matmuls of a head pair are issued back-to-back with
  no filler between: they sit in disjoint PE row-groups (rows 0-63 /
  64-127 via auto tile_position) and different PSUM banks, so they run
  CONCURRENTLY on the array (~-30us; a full-array filler in between
  serializes them).
- proj/cproj accumulation groups issue as unbroken 8/4-MM bursts (yield
  per group, not per MM) - single-PSUM-bank bursts keep HAM warm.
- q/k tiles bf16; cproj is fully bf16 (host-cast wproj + bf16 ot, so the
  po drain copies and normalize muls run on DVE's 2x 16-bit path). x and
  w_qkv stay f32r: converting the projections to bf16 measured +63us
  because every bf16 matmul emits an explicit Ldweights (~55ns apiece).
- HW-measured fixed cost per ACT/DVE/Pool instruction is ~0.5-1us, so op
  COUNT dominates the attention phase (phase ablation: drains+normalize
  were 208us of the 340us attention marginal cost). The po drain is one
  merged pair tile [128,2,N] (2 banks, ring 1 == old capacity) drained by
  ONE partition-shifted reciprocal [1,1024] + two ot copies at high
  scheduler priority; normalize (ONE pair broadcast + two aligned muls)
  is deferred 3 groups, SBUF-only, off the critical path. Splitting the
  exp in two measured +61us (ACT op overhead); moving muls to Pool
  measured +59us (Pool launch overhead); exp stays ONE [128,1024] ACT op
  per pair.
- PSUM rebalance (-12us): ps_st ring 1 (the bare ST->exp->ST loop is
  cheap; a 2-deep ring wastes 2 banks) frees 2 banks to double-buffer
  ps_o (ring 2 head-pair groups), taking the po drain off the PAV
  critical path. With the 1-deep pst ring LOOKAHEAD=2 beats 3 (and
  LOOKAHEAD=1 hard-crashes the NRT); pend>2 is the deferral optimum.
- engine balance (-16us): during attention ACT runs ONLY the 16 exps
  (exp latency gates the next ST via the 1-deep pst ring; st2/o1
  re-measured +50us WORSE even after this rebalance); cproj psum
  drains moved ACT->DVE, v-projection drains DVE->ACT (f32r ACT writes
  are full-rate; bf16 ACT writes are 2x slow, so qk/ot drains stay DVE;
  Pool launch overhead ~1us makes gpsimd muls a +59..65us regression).
- bf16 output partials (-8us): cproj drains write bf16 o_sb (2x DVE
  rate) and part is stored bf16 (half the DMA-out traffic); the host
  unshard sums the two head-half partials in f32. pend>3 deferral.
- qk drains ALL on ACT (-66us, the biggest single win): any qk copy on
  DVE head-of-line blocks the attention drains in DVE's strict FIFO ->
  pm/po ring stalls -> PE stalls. An even ACT/DVE split is +86us worse
  than all-ACT, despite ACT bf16 writes being 2x slow. Exp runs at high
  scheduler priority so it never queues behind the qk copies on ACT.
- same rule applied to cproj drains, DVE->ACT (-24us): DVE keeps ONLY
  the attention po-ring drains + normalize ops.
- two-stage deferred normalize (-10us): recip+Pool-broadcast issued one
  group deferred, the DVE muls TWO groups deferred - by then bc is long
  done, so the muls never WAIT inside DVE's FIFO (a waiting mul blocks
  the po drains queued behind it).
- TensorTensor operands must share a start partition (only unary Copy/
  Reciprocal may partition-shift); GPSIMD cannot read PSUM; DMA cannot
  read PSUM; ACT Reciprocal is blocked by bass.
"""

import numpy as np

B, N, A, DIM, H, D = 2, 512, 14, 1024, 16, 64
HL = 8            # heads per core
UNITS = 7         # (b, r) units per group
NCORES = 8
SCALE = np.float32(1.0 / np.sqrt(np.sqrt(D)))
VW = D + 1        # v width per head incl. ones column

_CACHE = {}


def _build_nc(units=UNITS, repeat=1, phases="QAC", qk_bias=False,
              v_bias=False):
    import concourse.bacc as bacc
    import concourse.tile as tile
    from concourse import mybir
    from concourse.bass import ts

    f32, f32r, bf16 = mybir.dt.float32, mybir.dt.float32r, mybir.dt.bfloat16
    AF = mybir.ActivationFunctionType

    nc = bacc.Bacc("TRN2", target_bir_lowering=False, debug=False,
                   num_devices=NCORES)
    xT = nc.dram_tensor("xT", [units, DIM, N], f32r, kind="ExternalInput")
    wqkv = nc.dram_tensor("wqkv", [DIM, 1024 + HL * D], f32r,
                          kind="ExternalInput")
    bqk = nc.dram_tensor("bqk", [1024], f32, kind="ExternalInput")
    bv = nc.dram_tensor("bv", [HL * VW], f32, kind="ExternalInput")
    wproj = nc.dram_tensor("wproj", [HL * D, DIM], bf16, kind="ExternalInput")
    part = nc.dram_tensor("part", [units, N, DIM], bf16, kind="ExternalOutput")

    import concourse.bass as bass

    def bcast_part(ap, p=128):
        return bass.AP(tensor=ap.tensor, offset=ap.offset,
                       ap=[[0, p]] + list(ap.ap))

    with tile.TileContext(nc) as tc:
        import contextlib
        with contextlib.ExitStack() as ctx:
            const = ctx.enter_context(tc.tile_pool(name="const", bufs=1))
            p_x = ctx.enter_context(tc.tile_pool(name="p_x", bufs=2))
            p_qk = ctx.enter_context(tc.tile_pool(name="p_qk", bufs=2))
            p_v = ctx.enter_context(tc.tile_pool(name="p_v", bufs=2))
            p_es = ctx.enter_context(tc.tile_pool(name="p_es", bufs=5))
            p_ot = ctx.enter_context(tc.tile_pool(name="p_ot", bufs=2))
            p_out = ctx.enter_context(tc.tile_pool(name="p_out", bufs=2))
            p_rc = ctx.enter_context(tc.tile_pool(name="p_rc", bufs=4))
            ps_mm = ctx.enter_context(
                tc.tile_pool(name="ps_mm", bufs=2, space="PSUM"))
            ps_st = ctx.enter_context(
                tc.tile_pool(name="ps_st", bufs=1, space="PSUM"))
            ps_o = ctx.enter_context(
                tc.tile_pool(name="ps_o", bufs=2, space="PSUM"))

            # ---- persistent weights ----
            wq_sb = const.tile([128, 8, 1024 + HL * D], f32r, tag="wqkv")
            _wq_r = wqkv[:].rearrange("(k p) c -> p k c", p=128)
            for k in range(8):
                nc.sync.dma_start(out=wq_sb[:, k, :], in_=_wq_r[:, k, :])
            wp_sb = const.tile([128, 4, DIM], bf16, tag="wproj")
            nc.sync.dma_start(
                out=wp_sb, in_=wproj[:].rearrange("(k p) c -> p k c", p=128))
            bqk_sb = const.tile([128, 8], f32, tag="bqk")
            nc.sync.dma_start(
                out=bqk_sb, in_=bqk[:].rearrange("(c p) -> p c", p=128))
            bv_sb = const.tile([128, HL * VW], f32, tag="bv")
            nc.sync.dma_start(out=bv_sb, in_=bcast_part(bv[:]))

            x_tiles, unit_state, ot_tiles = {}, {}, {}

            def emit_x(u):
                t = p_x.tile([128, 8, N], f32r, tag="x")
                nc.sync.dma_start(
                    out=t, in_=xT[u].rearrange("(k p) n -> p k n", p=128))
                x_tiles[u] = t

            def gen_proj(u):
                x_sb = x_tiles.pop(u)
                qk_sb = p_qk.tile([128, 8, N], bf16, tag="qk")
                v_sb = p_v.tile([128, 4, HL * VW], f32r, tag="v")
                unit_state[u] = (qk_sb, v_sb)

                # qk^T projection: [col, tok]; drains on ACT (bias is
                # per-partition here, so ACT's bias operand applies it free)
                # yield at GROUP granularity: the 8 accumulating matmuls of
                # one psum tile issue back-to-back (single-bank burst on the
                # PE, no interleaved bank cycling -> HAM stays warm)
                for ct in range(8):
                    pm = ps_mm.tile([128, N], f32, tag="mm")
                    for k in range(8):
                        nc.tensor.matmul(
                            pm, wq_sb[:, k, ts(ct, 128)], x_sb[:, k, :],
                            start=(k == 0), stop=(k == 7))
                    yield
                    with nc.allow_low_precision(reason="bf16 qk scores"):
                        if qk_bias:
                            nc.vector.tensor_scalar_add(
                                qk_sb[:, ct, :], pm, bqk_sb[:, ct:ct + 1])
                        else:
                            # ALL qk drains on ACT (-66us): any qk copy on
                            # DVE head-of-line blocks the attention drains
                            # in DVE's strict FIFO -> pm/po ring stalls ->
                            # PE stalls (an even ACT/DVE split is +86us)
                            nc.scalar.activation(
                                out=qk_sb[:, ct, :], in_=pm, func=AF.Copy)

                # v projection: [tok, lh*65+d]; 65th col per head = 1.0
                vv = v_sb.rearrange("p t (h w) -> p t h w", w=VW)
                bvv = bv_sb.rearrange("p (h w) -> p h w", w=VW)
                # ones columns for ALL tt in one DVE op (0-stride middle
                # dim repeats the initialized bv row; values are x0+1; on
                # ACT this measured ~410 vs ~406us on DVE)
                _b1 = bvv[:, :, D]
                nc.vector.tensor_scalar(
                    out=vv[:, :, :, D],
                    in0=bass.AP(tensor=_b1.tensor, offset=_b1.offset,
                                ap=[list(_b1.ap)[0], [0, 4]] +
                                   list(_b1.ap)[1:]),
                    scalar1=0.0, scalar2=1.0,
                    op0=mybir.AluOpType.mult,
                    op1=mybir.AluOpType.add)
                for tt in range(4):
                    pv = ps_mm.tile([128, N], f32, tag="mm")
                    pvv = pv.rearrange("p (h d) -> p h d", d=D)
                    for k in range(8):
                        nc.tensor.matmul(
                            pv, x_sb[:, k, ts(tt, 128)],
                            wq_sb[:, k, 1024:1024 + HL * D],
                            start=(k == 0), stop=(k == 7))
                    yield
                    with nc.allow_low_precision(reason="f32r v tile"):
                        if v_bias:
                            nc.vector.tensor_add(
                                out=vv[:, tt, :, 0:D], in0=pvv,
                                in1=bvv[:, :, 0:D])
                        else:
                            # ACT copy (f32r write is full-rate unlike
                            # bf16) - keeps 4 ops off the congested DVE
                            nc.scalar.activation(
                                out=vv[:, tt, :, 0:D], in_=pvv,
                                func=AF.Copy)

            def gen_attn(u):
                qk_sb, v_sb = unit_state.pop(u)
                if "nopav" in phases or "nodrain" in phases:
                    ot_sb = qk_sb      # ablations never write ot
                else:
                    ot_sb = p_ot.tile([128, 4, N], bf16, tag="ot")
                ot_tiles[u] = ot_sb
                pend = []

                def normalize_bc(c, rc):
                    # stage 1 of deferred normalize: Pool broadcast only
                    bc = p_rc.tile([128, 2, N], f32r, tag="bc")
                    nc.gpsimd.partition_broadcast(
                        bc.rearrange("p a b -> p (a b)"),
                        rc.rearrange("p a b -> p (a b)")[0:1, :])
                    return bc

                def normalize_mul(c, bc):
                    # stage 2, two groups later: by now the broadcast is
                    # long done, so these DVE muls never WAIT inside the
                    # strict FIFO (a waiting mul head-of-line blocks the
                    # po drains behind it)
                    nc.vector.tensor_mul(
                        out=ot_sb[0:64, c, :],
                        in0=ot_sb[0:64, c, :], in1=bc[0:64, 0, :])
                    nc.vector.tensor_mul(
                        out=ot_sb[64:128, c, :],
                        in0=ot_sb[64:128, c, :], in1=bc[64:128, 1, :])

                # pair-merged: heads (2c, 2c+1) share a 2-bank score psum
                # tile and ONE exp instruction per (c, jt) -> halves the
                # PE->ACT->PE round trips and ACT instruction overhead
                pairs = [(c, jt) for c in range(HL // 2) for jt in range(4)]
                pos, ess = {}, {}
                pend2 = []

                def gen_st_pair(c, jt, st_weight=1):
                    # K=64 scores straight from the qk tile: head lh has its
                    # q cols in qk[:, lh//2] and k cols in qk[:, 4+lh//2],
                    # both on partition rows (lh%2)*64..+64.
                    # The two matmuls sit in disjoint PE row-groups (rows
                    # 0-63 / 64-127 via auto tile_position) and write
                    # different PSUM banks, so issued back-to-back they run
                    # CONCURRENTLY on the array (~512 cycles for the pair,
                    # not 1024). No yield between them: a full-array filler
                    # matmul in the middle would serialize the pair.
                    pst = ps_st.tile([128, 2, N], f32, tag="st")
                    for h01 in range(2):
                        hp = h01 * 64
                        nc.tensor.matmul(
                            pst[:, h01, :],
                            qk_sb[hp:hp + 64, 4 + c, ts(jt, 128)],
                            qk_sb[hp:hp + 64, c, :], start=True, stop=True)
                    yield st_weight
                    es_t = p_es.tile([128, 2, N], f32r, tag="es")
                    with nc.allow_low_precision(reason="bf16 softmax"):
                        if "noexp" in phases:  # debug: DVE copy, no ACT
                            nc.vector.tensor_copy(
                                out=es_t.rearrange("p a b -> p (a b)"),
                                in_=pst.rearrange("p a b -> p (a b)"))
                        else:
                            # ONE exp per pair: ACT instructions cost ~0.5us
                            # fixed on HW (splitting this in two measured
                            # +61us total). High priority: exps gate the
                            # 1-deep pst ring and must not queue behind the
                            # qk drain copies that now share ACT
                            with tc.high_priority(offset=64):
                                nc.scalar.activation(
                                    out=es_t.rearrange("p a b -> p (a b)"),
                                    in_=pst.rearrange("p a b -> p (a b)"),
                                    func=AF.Exp)
                    ess[(c, jt)] = es_t

                def gen_pav_pair(c, jt):
                    es_t = ess.pop((c, jt))
                    if "nopav" in phases:  # ablation: ST+exp only
                        return
                    if jt == 0:
                        # ONE 2-bank psum tile per head pair (same capacity
                        # as two 1-bank tiles): pair drain is then 1 merged
                        # recip + 2 copies instead of 2 recips + 2 copies
                        po_g = ps_o.tile([128, 2, N], f32, tag="o")
                        pos[c] = po_g
                    po = pos[c]
                    for h01 in range(2):
                        lh = 2 * c + h01
                        nc.tensor.matmul(
                            po[0:VW, h01, :],
                            v_sb[:, jt, lh * VW:(lh + 1) * VW],
                            es_t[:, h01, :], start=(jt == 0), stop=(jt == 3))
                        yield 1
                    if jt == 3:
                        po = pos.pop(c)
                        if "nodrain" in phases:  # ablation: no po drain
                            return
                        rc = p_rc.tile([1, 2, N], f32r, tag="rc")
                        with nc.allow_low_precision(
                                reason="f32r softmax recip"), \
                                tc.high_priority(offset=64):
                            # high priority: the scheduler orders these DVE
                            # ops ahead of queued proj/v drains so the po
                            # psum ring frees asap (the ring gates PAV)
                            # partition-shifted unary ops are legal (Copy/
                            # Reciprocal); only TensorTensor must align
                            nc.vector.reciprocal(
                                out=rc.rearrange("p a b -> p (a b)"),
                                in_=po[64:65, :, :].rearrange(
                                    "p a b -> p (a b)"))
                            nc.vector.tensor_copy(
                                out=ot_sb[0:64, c, :], in_=po[0:64, 0, :])
                            nc.vector.tensor_copy(
                                out=ot_sb[64:128, c, :], in_=po[0:64, 1, :])
                        if "nonorm" not in phases:
                            pend.append((c, rc))
                        if len(pend) > 1:
                            c2, rc2 = pend.pop(0)
                            pend2.append((c2, normalize_bc(c2, rc2)))
                        if len(pend2) > 1:
                            normalize_mul(*pend2.pop(0))

                LOOKAHEAD = 2
                for s in range(len(pairs) + LOOKAHEAD):
                    if s < len(pairs):
                        yield from gen_st_pair(*pairs[s])
                    if s >= LOOKAHEAD:
                        yield from gen_pav_pair(*pairs[s - LOOKAHEAD])
                while pend:
                    c2, rc2 = pend.pop(0)
                    pend2.append((c2, normalize_bc(c2, rc2)))
                while pend2:
                    normalize_mul(*pend2.pop(0))

            def gen_cproj(u):
                ot_sb = ot_tiles.pop(u)
                if "C" not in phases:
                    # debug: dump ot (or qk when attn is skipped) raw; both
                    # flatten+bitcast to [128, 2048] f32
                    flat = ot_sb.rearrange("p a b -> p (a b)").bitcast(f32)
                    for tt in range(4):
                        nc.sync.dma_start(
                            out=part[u, ts(tt, 128), :],
                            in_=flat[:, ts(tt % 2, 1024)])
                    return
                for tt in range(4):
                    o_sb = p_out.tile([128, DIM], bf16, tag="out")
                    for eh in range(2):
                        pc = ps_mm.tile([128, N], f32, tag="mm")
                        for ct in range(4):
                            nc.tensor.matmul(
                                pc, ot_sb[:, ct, ts(tt, 128)],
                                wp_sb[:, ct, eh * 512:(eh + 1) * 512],
                                start=(ct == 0), stop=(ct == 3))
                        yield
                        # ACT like the qk drains: any non-attention copy in
                        # DVE's strict FIFO can head-of-line block the po
                        # drains. bf16 out: ACT bf16 writes are 2x slow but
                        # f32r partials re-measured ~421 vs ~406-417us here
                        # (the halved store traffic wins).
                        with nc.allow_low_precision(reason="bf16 partials"):
                            nc.scalar.activation(
                                out=o_sb[:, eh * 512:(eh + 1) * 512],
                                in_=pc, func=AF.Copy)
                    nc.sync.dma_start(
                        out=part[u, ts(tt, 128), :], in_=o_sb)

            def body():
                from itertools import chain
                emit_x(0)
                if units > 1:
                    emit_x(1)
                for _ in gen_proj(0):
                    pass
                def attn_or_skip(u):
                    if "A" in phases:
                        yield from gen_attn(u)
                    else:
                        qk_sb, v_sb = unit_state.pop(u)
                        ot_tiles[u] = qk_sb
                for s in range(units):
                    if s + 2 < units:
                        emit_x(s + 2)
                    fillers = []
                    nf = 0
                    if s + 1 < units:
                        fillers.append(gen_proj(s + 1))
                        nf += 12
                    if s >= 1:
                        fillers.append(gen_cproj(s - 1))
                        nf += 8
                    filler = chain(*fillers)
                    # spread the filler matmuls evenly over the attention
                    # yields (48 per unit when "A" in phases) so the PE sees
                    # constant pressure instead of 2-per-yield + a tail dump
                    ny = 48 if "A" in phases else 1
                    done = pulled = 0
                    for w in attn_or_skip(s):
                        done += w
                        want = (nf * done) // ny
                        while pulled < want:
                            if next(filler, None) is None:
                                pulled = nf
                                break
                            pulled += 1
                    for _ in filler:
                        pass
                for _ in gen_cproj(units - 1):
                    pass

            nbody = 2 if "u2" in phases else 1
            if repeat == 1:
                for _ in range(nbody):
                    body()
            else:
                with tc.For_i(0, repeat, 1):
                    for _ in range(nbody):
                        body()

    nc.compile()
    return nc


def _make_runner(nc, n_cores=NCORES, donate=True):
    """Persistent jitted SPMD runner (mirrors bass2jax.run_bass_via_pjrt)."""
    import jax
    from jax.sharding import Mesh, PartitionSpec
    from jax.experimental.shard_map import shard_map
    from concourse import bass2jax
    from concourse import mybir as mb

    bass2jax.install_neuronx_cc_hook()
    pn = nc.partition_id_tensor.name if nc.partition_id_tensor else None
    in_names, out_names, out_avals, out_shapes = [], [], [], []
    for alloc in nc.m.functions[0].allocations:
        if not isinstance(alloc, mb.MemoryLocationSet):
            continue
        name = alloc.memorylocations[0].name
        if alloc.kind == "ExternalInput":
            if name != pn:
                in_names.append(name)
        elif alloc.kind == "ExternalOutput":
            shape = tuple(alloc.tensor_shape)
            dtype = mb.dt.np(alloc.dtype)
            out_names.append(name)
            out_avals.append(jax.core.ShapedArray(shape, dtype))
            out_shapes.append((shape, dtype))
    n_params = len(in_names)
    n_outs = len(out_names)
    all_in = list(in_names) + list(out_names) + ([pn] if pn else [])

    def _body(*args):
        ops = list(args)
        if pn:
            ops.append(bass2jax.partition_id_tensor())
        return tuple(bass2jax._bass_exec_p.bind(
            *ops, out_avals=tuple(out_avals), in_names=tuple(all_in),
            out_names=tuple(out_names), lowering_input_output_aliases=(),
            sim_require_finite=True, sim_require_nnan=True, nc=nc))

    devices = jax.devices()[:n_cores]
    mesh = Mesh(np.asarray(devices), ("core",))
    specs = (PartitionSpec("core"),)
    fn = jax.jit(
        shard_map(_body, mesh=mesh, in_specs=specs * (n_params + n_outs),
                  out_specs=specs * n_outs, check_rep=False),
        donate_argnums=tuple(range(n_params, n_params + n_outs)) if donate else (),
        keep_unused=True)

    def run(in_maps):
        per_core = [[np.asarray(m[name]) for name in in_names] for m in in_maps]
        concat_in = [np.concatenate([per_core[c][i] for c in range(n_cores)],
                                    axis=0) for i in range(n_params)]
        concat_zeros = [np.zeros((n_cores * s[0], *s[1:]), d)
                        for (s, d) in out_shapes]
        import jax as _jax
        out_arrs = _jax.block_until_ready(fn(*concat_in, *concat_zeros))
        return [
            {name: np.asarray(out_arrs[i]).reshape(n_cores, *out_shapes[i][0])[c]
             for i, name in enumerate(out_names)}
            for c in range(n_cores)
        ]

    run.jit_fn = fn
    run.in_names = in_names
    run.out_names = out_names
    run.out_shapes = out_shapes
    run.n_cores = n_cores
    return run


def _unit_groups():
    units = [(b, r) for b in range(B) for r in range(A)]
    return [units[g * UNITS:(g + 1) * UNITS] for g in range(4)]


def shard_inputs(x, w_qkv, b_qkv, w_proj, b_proj):
    groups = _unit_groups()
    w4 = w_qkv.reshape(DIM, H, 3, D)
    b4 = b_qkv.reshape(H, 3, D)
    in_maps = []
    for c in range(NCORES):
        g, hh = c // 2, c % 2
        heads = list(range(hh * HL, (hh + 1) * HL))
        xT = np.ascontiguousarray(
            np.stack([x[b, :, r, :].T for (b, r) in groups[g]])
        ).astype(np.float32)
        wq = w4[:, heads, 0, :].reshape(DIM, HL * D) * SCALE
        wk = w4[:, heads, 1, :].reshape(DIM, HL * D) * SCALE
        wv = w4[:, heads, 2, :].reshape(DIM, HL * D)
        wqkv_c = np.ascontiguousarray(
            np.concatenate([wq, wk, wv], axis=1)).astype(np.float32)
        bq = (b4[heads, 0, :].reshape(HL * D) * SCALE)
        bk = (b4[heads, 1, :].reshape(HL * D) * SCALE)
        bvv = np.concatenate([b4[heads, 2, :], np.ones((HL, 1), np.float32)],
                             axis=1).reshape(HL * VW)
        in_maps.append({
            "xT": xT,
            "wqkv": wqkv_c,
            "bqk": np.concatenate([bq, bk]).astype(np.float32),
            "bv": bvv.astype(np.float32),
            "wproj": np.ascontiguousarray(
                w_proj[hh * HL * D:(hh + 1) * HL * D, :]).astype(
                    __import__("ml_dtypes").bfloat16),
        })
    return in_maps


def unshard(results, b_proj):
    groups = _unit_groups()
    out = np.zeros((B, N, A, DIM), np.float32)
    for g in range(4):
        s = (results[2 * g]["part"].astype(np.float32)
             + results[2 * g + 1]["part"].astype(np.float32))
        for idx, (b, r) in enumerate(groups[g]):
            out[b, :, r, :] = s[idx]
    return out + b_proj.astype(np.float32)


def get_runner(qk_bias=False, v_bias=False):
    key = ("runner", qk_bias, v_bias)
    if key not in _CACHE:
        nc = _build_nc(qk_bias=qk_bias, v_bias=v_bias)
        _CACHE[key] = _make_runner(nc)
    return _CACHE[key]


def kernel(x, w_qkv, b_qkv, w_proj, b_proj):
    x = np.asarray(x)
    w_qkv = np.asarray(w_qkv)
    b_qkv = np.asarray(b_qkv)
    w_proj = np.asarray(w_proj)
    b_proj = np.asarray(b_proj)
    run = get_runner(qk_bias=bool(np.any(b_qkv[:2048])),
                     v_bias=bool(np.any(b_qkv[2048:])))
    in_maps = shard_inputs(x, w_qkv, b_qkv, w_proj, b_proj)
    results = run(in_maps)
    return unshard(results, b_proj)



# revision 78
# speedup vs baseline: 1.0456x; 1.0456x over previous
"""Trainium2 Bass kernel for nn_Attention_v4 (sparse per-atom attention).

Reference computation (fp32):
    x:[2,512,14,1024] -> qkv = x@w_qkv+b_qkv -> per (b, r=atom, head)
    attention over the n=512 axis -> out @ w_proj + b_proj.

Sharding (8 cores): 4 groups x 7 (b,r)-units data-parallel, x 2 head-halves
tensor-parallel. Each core computes, for its 7 units and its 8 heads:
QKV^T projection, attention, and a partial c_proj (contraction over its 512
of the 1024 hd rows). Host unshard sums the two head-half partials and adds
b_proj.

Schedule (all engine/latency choices HW-measured on these cores; 603us
baseline -> ~406-417us through the changes below):
- unit-level software pipeline: stage s emits dma_x(s+2) then interleaves
  [ attn(s) : proj(s+1) + cproj(s-1) ] filler matmul GROUPS spread evenly
  over the 48 attention yields (3 head-pairs of exp->PAV lookahead).
- the two K=64 score matmuls of a head pair are issued back-to-back with
  no filler between: they sit in disjoint PE row-groups (rows 0-63 /
  64-127 via auto tile_position) and different PSUM banks, so they run
  CONCURRENTLY on the array (~-30us; a full-array filler in between
  serializes them).
- proj/cproj accumulation groups issue as unbroken 8/4-MM bursts (yield
  per group, not per MM) - single-PSUM-bank bursts keep HAM warm.
- q/k tiles bf16; cproj is fully bf16 (host-cast wproj + bf16 ot, so the
  po drain copies and normalize muls run on DVE's 2x 16-bit path). x and
  w_qkv stay f32r: converting the projections to bf16 measured +63us
  because every bf16 matmul emits an explicit Ldweights (~55ns apiece).
- HW-measured fixed cost per ACT/DVE/Pool instruction is ~0.5-1us, so op
  COUNT dominates the attention phase (phase ablation: drains+normalize
  were 208us of the 340us attention marginal cost). The po drain is one
  merged pair tile [128,2,N] (2 banks, ring 1 == old capacity) drained by
  ONE partition-shifted reciprocal [1,1024] + two ot copies at high
  scheduler priority; normalize (ONE pair broadcast + two aligned muls)
  is deferred 3 groups, SBUF-only, off the critical path. Splitting the
  exp in two measured +61us (ACT op overhead); moving muls to Pool
  measured +59us (Pool launch overhead); exp stays ONE [128,1024] ACT op
  per pair.
- PSUM rebalance (-12us): ps_st ring 1 (the bare ST->exp->ST loop is
  cheap; a 2-deep ring wastes 2 banks) frees 2 banks to double-buffer
  ps_o (ring 2 head-pair groups), taking the po drain off the PAV
  critical path. With the 1-deep pst ring LOOKAHEAD=2 beats 3 (and
  LOOKAHEAD=1 hard-crashes the NRT); pend>2 is the deferral optimum.
- engine balance (-16us): during attention ACT runs ONLY the 16 exps
  (exp latency gates the next ST via the 1-deep pst ring; st2/o1
  re-measured +50us WORSE even after this rebalance); cproj psum
  drains moved ACT->DVE, v-projection drains DVE->ACT (f32r ACT writes
  are full-rate; bf16 ACT writes are 2x slow, so qk/ot drains stay DVE;
  Pool launch overhead ~1us makes gpsimd muls a +59..65us regression).
- bf16 output partials (-8us): cproj drains write bf16 o_sb (2x DVE
  rate) and part is stored bf16 (half the DMA-out traffic); the host
  unshard sums the two head-half partials in f32. pend>3 deferral.
- qk drains ALL on ACT (-66us, the biggest single win): any qk copy on
  DVE head-of-line blocks the attention drains in DVE's strict FIFO ->
  pm/po ring stalls -> PE stalls. An even ACT/DVE split is +86us worse
  than all-ACT, despite ACT bf16 writes being 2x slow. Exp runs at high
  scheduler priority so it never queues behind the qk copies on ACT.
- same rule applied to cproj drains, DVE->ACT (-24us): DVE keeps ONLY
  the attention po-ring drains + normalize ops.
- two-stage deferred normalize (-10us): recip+Pool-broadcast issued one
  group deferred, the DVE muls TWO groups deferred - by then bc is long
  done, so the muls never WAIT inside DVE's FIFO (a waiting mul blocks
  the po drains queued behind it).
- TensorTensor operands must share a start partition (only unary Copy/
  Reciprocal may partition-shift); GPSIMD cannot read PSUM; DMA cannot
  read PSUM; ACT Reciprocal is blocked by bass.
"""

import numpy as np

B, N, A, DIM, H, D = 2, 512, 14, 1024, 16, 64
HL = 8            # heads per core
UNITS = 7         # (b, r) units per group
NCORES = 8
SCALE = np.float32(1.0 / np.sqrt(np.sqrt(D)))
VW = D + 1        # v width per head incl. ones column

_CACHE = {}


def _build_nc(units=UNITS, repeat=1, phases="QAC", qk_bias=False,
              v_bias=False):
    import concourse.bacc as bacc
    import concourse.tile as tile
    from concourse import mybir
    from concourse.bass import ts

    f32, f32r, bf16 = mybir.dt.float32, mybir.dt.float32r, mybir.dt.bfloat16
    AF = mybir.ActivationFunctionType

    nc = bacc.Bacc("TRN2", target_bir_lowering=False, debug=False,
                   num_devices=NCORES)
    xT = nc.dram_tensor("xT", [units, DIM, N], f32r, kind="ExternalInput")
    wqkv = nc.dram_tensor("wqkv", [DIM, 1024 + HL * D], f32r,
                          kind="ExternalInput")
    bqk = nc.dram_tensor("bqk", [1024], f32, kind="ExternalInput")
    bv = nc.dram_tensor("bv", [HL * VW], f32, kind="ExternalInput")
    wproj = nc.dram_tensor("wproj", [HL * D, DIM], bf16, kind="ExternalInput")
    part = nc.dram_tensor("part", [units, N, DIM], bf16, kind="ExternalOutput")

    import concourse.bass as bass

    def bcast_part(ap, p=128):
        return bass.AP(tensor=ap.tensor, offset=ap.offset,
                       ap=[[0, p]] + list(ap.ap))

    with tile.TileContext(nc) as tc:
        import contextlib
        with contextlib.ExitStack() as ctx:
            const = ctx.enter_context(tc.tile_pool(name="const", bufs=1))
            p_x = ctx.enter_context(tc.tile_pool(name="p_x", bufs=2))
            p_qk = ctx.enter_context(tc.tile_pool(name="p_qk", bufs=2))
            p_v = ctx.enter_context(tc.tile_pool(name="p_v", bufs=2))
            p_es = ctx.enter_context(tc.tile_pool(name="p_es", bufs=5))
            p_ot = ctx.enter_context(tc.tile_pool(name="p_ot", bufs=2))
            p_out = ctx.enter_context(tc.tile_pool(name="p_out", bufs=4))
            p_rc = ctx.enter_context(tc.tile_pool(name="p_rc", bufs=4))
            ps_mm = ctx.enter_context(
                tc.tile_pool(name="ps_mm", bufs=2, space="PSUM"))
            ps_st = ctx.enter_context(
                tc.tile_pool(name="ps_st", bufs=1, space="PSUM"))
            ps_o = ctx.enter_context(
                tc.tile_pool(name="ps_o", bufs=2, space="PSUM"))

            # ---- persistent weights ----
            wq_sb = const.tile([128, 8, 1024 + HL * D], f32r, tag="wqkv")
            _wq_r = wqkv[:].rearrange("(k p) c -> p k c", p=128)
            for k in range(8):
                nc.sync.dma_start(out=wq_sb[:, k, :], in_=_wq_r[:, k, :])
            wp_sb = const.tile([128, 4, DIM], bf16, tag="wproj")
            nc.sync.dma_start(
                out=wp_sb, in_=wproj[:].rearrange("(k p) c -> p k c", p=128))
            bqk_sb = const.tile([128, 8], f32, tag="bqk")
            nc.sync.dma_start(
                out=bqk_sb, in_=bqk[:].rearrange("(c p) -> p c", p=128))
            bv_sb = const.tile([128, HL * VW], f32, tag="bv")
            nc.sync.dma_start(out=bv_sb, in_=bcast_part(bv[:]))

            x_tiles, unit_state, ot_tiles = {}, {}, {}

            def emit_x(u):
                t = p_x.tile([128, 8, N], f32r, tag="x")
                nc.sync.dma_start(
                    out=t, in_=xT[u].rearrange("(k p) n -> p k n", p=128))
                x_tiles[u] = t

            def gen_proj(u):
                x_sb = x_tiles.pop(u)
                qk_sb = p_qk.tile([128, 8, N], bf16, tag="qk")
                v_sb = p_v.tile([128, 4, HL * VW], f32r, tag="v")
                unit_state[u] = (qk_sb, v_sb)

                # qk^T projection: [col, tok]; drains on ACT (bias is
                # per-partition here, so ACT's bias operand applies it free)
                # yield at GROUP granularity: the 8 accumulating matmuls of
                # one psum tile issue back-to-back (single-bank burst on the
                # PE, no interleaved bank cycling -> HAM stays warm)
                for ct in range(8):
                    pm = ps_mm.tile([128, N], f32, tag="mm")
                    for k in range(8):
                        nc.tensor.matmul(
                            pm, wq_sb[:, k, ts(ct, 128)], x_sb[:, k, :],
                            start=(k == 0), stop=(k == 7))
                    yield
                    with nc.allow_low_precision(reason="bf16 qk scores"):
                        if qk_bias:
                            nc.vector.tensor_scalar_add(
                                qk_sb[:, ct, :], pm, bqk_sb[:, ct:ct + 1])
                        else:
                            # ALL qk drains on ACT (-66us): any qk copy on
                            # DVE head-of-line blocks the attention drains
                            # in DVE's strict FIFO -> pm/po ring stalls ->
                            # PE stalls (an even ACT/DVE split is +86us)
                            nc.scalar.activation(
                                out=qk_sb[:, ct, :], in_=pm, func=AF.Copy)

                # v projection: [tok, lh*65+d]; 65th col per head = 1.0
                vv = v_sb.rearrange("p t (h w) -> p t h w", w=VW)
                bvv = bv_sb.rearrange("p (h w) -> p h w", w=VW)
                # ones columns for ALL tt in one DVE op (0-stride middle
                # dim repeats the initialized bv row; values are x0+1; on
                # ACT this measured ~410 vs ~406us on DVE)
                _b1 = bvv[:, :, D]
                nc.vector.tensor_scalar(
                    out=vv[:, :, :, D],
                    in0=bass.AP(tensor=_b1.tensor, offset=_b1.offset,
                                ap=[list(_b1.ap)[0], [0, 4]] +
                                   list(_b1.ap)[1:]),
                    scalar1=0.0, scalar2=1.0,
                    op0=mybir.AluOpType.mult,
                    op1=mybir.AluOpType.add)
                for tt in range(4):
                    pv = ps_mm.tile([128, N], f32, tag="mm")
                    pvv = pv.rearrange("p (h d) -> p h d", d=D)
                    for k in range(8):
                        nc.tensor.matmul(
                            pv, x_sb[:, k, ts(tt, 128)],
                            wq_sb[:, k, 1024:1024 + HL * D],
                            start=(k == 0), stop=(k == 7))
                    yield
                    with nc.allow_low_precision(reason="f32r v tile"):
                        if v_bias:
                            nc.vector.tensor_add(
                                out=vv[:, tt, :, 0:D], in0=pvv,
                                in1=bvv[:, :, 0:D])
                        else:
                            # ACT copy (f32r write is full-rate unlike
                            # bf16) - keeps 4 ops off the congested DVE
                            nc.scalar.activation(
                                out=vv[:, tt, :, 0:D], in_=pvv,
                                func=AF.Copy)

            def gen_attn(u):
                qk_sb, v_sb = unit_state.pop(u)
                if "nopav" in phases or "nodrain" in phases:
                    ot_sb = qk_sb      # ablations never write ot
                else:
                    ot_sb = p_ot.tile([128, 4, N], bf16, tag="ot")
                ot_tiles[u] = ot_sb
                pend = []

                def normalize_bc(c, rc):
                    # stage 1 of deferred normalize: Pool broadcast only
                    bc = p_rc.tile([128, 2, N], f32r, tag="bc")
                    nc.gpsimd.partition_broadcast(
                        bc.rearrange("p a b -> p (a b)"),
                        rc.rearrange("p a b -> p (a b)")[0:1, :])
                    return bc

                def normalize_mul(c, bc):
                    # stage 2, two groups later: by now the broadcast is
                    # long done, so these DVE muls never WAIT inside the
                    # strict FIFO (a waiting mul head-of-line blocks the
                    # po drains behind it)
                    nc.vector.tensor_mul(
                        out=ot_sb[0:64, c, :],
                        in0=ot_sb[0:64, c, :], in1=bc[0:64, 0, :])
                    nc.vector.tensor_mul(
                        out=ot_sb[64:128, c, :],
                        in0=ot_sb[64:128, c, :], in1=bc[64:128, 1, :])

                # pair-merged: heads (2c, 2c+1) share a 2-bank score psum
                # tile and ONE exp instruction per (c, jt) -> halves the
                # PE->ACT->PE round trips and ACT instruction overhead
                pairs = [(c, jt) for c in range(HL // 2) for jt in range(4)]
                pos, ess = {}, {}
                pend2 = []

                def gen_st_pair(c, jt, st_weight=1):
                    # K=64 scores straight from the qk tile: head lh has its
                    # q cols in qk[:, lh//2] and k cols in qk[:, 4+lh//2],
                    # both on partition rows (lh%2)*64..+64.
                    # The two matmuls sit in disjoint PE row-groups (rows
                    # 0-63 / 64-127 via auto tile_position) and write
                    # different PSUM banks, so issued back-to-back they run
                    # CONCURRENTLY on the array (~512 cycles for the pair,
                    # not 1024). No yield between them: a full-array filler
                    # matmul in the middle would serialize the pair.
                    pst = ps_st.tile([128, 2, N], f32, tag="st")
                    for h01 in range(2):
                        hp = h01 * 64
                        nc.tensor.matmul(
                            pst[:, h01, :],
                            qk_sb[hp:hp + 64, 4 + c, ts(jt, 128)],
                            qk_sb[hp:hp + 64, c, :], start=True, stop=True)
                    yield st_weight
                    es_t = p_es.tile([128, 2, N], f32r, tag="es")
                    with nc.allow_low_precision(reason="bf16 softmax"):
                        if "noexp" in phases:  # debug: DVE copy, no ACT
                            nc.vector.tensor_copy(
                                out=es_t.rearrange("p a b -> p (a b)"),
                                in_=pst.rearrange("p a b -> p (a b)"))
                        else:
                            # ONE exp per pair: ACT instructions cost ~0.5us
                            # fixed on HW (splitting this in two measured
                            # +61us total). High priority: exps gate the
                            # 1-deep pst ring and must not queue behind the
                            # qk drain copies that now share ACT
                            with tc.high_priority(offset=64):
                                nc.scalar.activation(
                                    out=es_t.rearrange("p a b -> p (a b)"),
                                    in_=pst.rearrange("p a b -> p (a b)"),
                                    func=AF.Exp)
                    ess[(c, jt)] = es_t

                def gen_pav_pair(c, jt):
                    es_t = ess.pop((c, jt))
                    if "nopav" in phases:  # ablation: ST+exp only
                        return
                    if jt == 0:
                        # ONE 2-bank psum tile per head pair (same capacity
                        # as two 1-bank tiles): pair drain is then 1 merged
                        # recip + 2 copies instead of 2 recips + 2 copies
                        po_g = ps_o.tile([128, 2, N], f32, tag="o")
                        pos[c] = po_g
                    po = pos[c]
                    for h01 in range(2):
                        lh = 2 * c + h01
                        nc.tensor.matmul(
                            po[0:VW, h01, :],
                            v_sb[:, jt, lh * VW:(lh + 1) * VW],
                            es_t[:, h01, :], start=(jt == 0), stop=(jt == 3))
                        yield 1
                    if jt == 3:
                        po = pos.pop(c)
                        if "nodrain" in phases:  # ablation: no po drain
                            return
                        rc = p_rc.tile([1, 2, N], f32r, tag="rc")
                        with nc.allow_low_precision(
                                reason="f32r softmax recip"), \
                                tc.high_priority(offset=64):
                            # high priority: the scheduler orders these DVE
                            # ops ahead of queued proj/v drains so the po
                            # psum ring frees asap (the ring gates PAV)
                            # partition-shifted unary ops are legal (Copy/
                            # Reciprocal); only TensorTensor must align
                            nc.vector.reciprocal(
                                out=rc.rearrange("p a b -> p (a b)"),
                                in_=po[64:65, :, :].rearrange(
                                    "p a b -> p (a b)"))
                            nc.vector.tensor_copy(
                                out=ot_sb[0:64, c, :], in_=po[0:64, 0, :])
                            nc.vector.tensor_copy(
                                out=ot_sb[64:128, c, :], in_=po[0:64, 1, :])
                        if "nonorm" not in phases:
                            pend.append((c, rc))
                        if len(pend) > 1:
                            c2, rc2 = pend.pop(0)
                            pend2.append((c2, normalize_bc(c2, rc2)))
                        if len(pend2) > 1:
                            normalize_mul(*pend2.pop(0))

                LOOKAHEAD = 2
                for s in range(len(pairs) + LOOKAHEAD):
                    if s < len(pairs):
                        yield from gen_st_pair(*pairs[s])
                    if s >= LOOKAHEAD:
                        yield from gen_pav_pair(*pairs[s - LOOKAHEAD])
                while pend:
                    c2, rc2 = pend.pop(0)
                    pend2.append((c2, normalize_bc(c2, rc2)))
                while pend2:
                    normalize_mul(*pend2.pop(0))

            def gen_cproj(u):
                ot_sb = ot_tiles.pop(u)
                if "C" not in phases:
                    # debug: dump ot (or qk when attn is skipped) raw; both
                    # flatten+bitcast to [128, 2048] f32
                    flat = ot_sb.rearrange("p a b -> p (a b)").bitcast(f32)
                    for tt in range(4):
                        nc.sync.dma_start(
                            out=part[u, ts(tt, 128), :],
                            in_=flat[:, ts(tt % 2, 1024)])
                    return
                for tt in range(4):
                    o_sb = p_out.tile([128, DIM], bf16, tag="out")
                    for eh in range(2):
                        pc = ps_mm.tile([128, N], f32, tag="mm")
                        for ct in range(4):
                            nc.tensor.matmul(
                                pc, ot_sb[:, ct, ts(tt, 128)],
                                wp_sb[:, ct, eh * 512:(eh + 1) * 512],
                                start=(ct == 0), stop=(ct == 3))
                        yield
                        # ACT like the qk drains: any non-attention copy in
                        # DVE's strict FIFO can head-of-line block the po
                        # drains. bf16 out: ACT bf16 writes are 2x slow but
                        # f32r partials re-measured ~421 vs ~406-417us here
                        # (the halved store traffic wins).
                        with nc.allow_low_precision(reason="bf16 partials"):
                            nc.scalar.activation(
                                out=o_sb[:, eh * 512:(eh + 1) * 512],
                                in_=pc, func=AF.Copy)
                    nc.sync.dma_start(
                        out=part[u, ts(tt, 128), :], in_=o_sb)

            def body():
                from itertools import chain
                emit_x(0)
                if units > 1:
                    emit_x(1)
                for _ in gen_proj(0):
                    pass
                def attn_or_skip(u):
                    if "A" in phases:
                        yield from gen_attn(u)
                    else:
                        qk_sb, v_sb = unit_state.pop(u)
                        ot_tiles[u] = qk_sb
                for s in range(units):
                    if s + 2 < units:
                        emit_x(s + 2)
                    fillers = []
                    nf = 0
                    if s + 1 < units:
                        fillers.append(gen_proj(s + 1))
                        nf += 12
                    if s >= 1:
                        fillers.append(gen_cproj(s - 1))
                        nf += 8
                    filler = chain(*fillers)
                    # spread the filler matmuls evenly over the attention
                    # yields (48 per unit when "A" in phases) so the PE sees
                    # constant pressure instead of 2-per-yield + a tail dump
                    ny = 48 if "A" in phases else 1
                    done = pulled = 0
                    for w in attn_or_skip(s):
                        done += w
                        want = (nf * done) // ny
                        while pulled < want:
                            if next(filler, None) is None:
                                pulled = nf
                                break
                            pulled += 1
                    for _ in filler:
                        pass
                for _ in gen_cproj(units - 1):
                    pass

            nbody = 2 if "u2" in phases else 1
            if repeat == 1:
                for _ in range(nbody):
                    body()
            else:
                with tc.For_i(0, repeat, 1):
                    for _ in range(nbody):
                        body()

    nc.compile()
    return nc


def _make_runner(nc, n_cores=NCORES, donate=True):
    """Persistent jitted SPMD runner (mirrors bass2jax.run_bass_via_pjrt)."""
    import jax
    from jax.sharding import Mesh, PartitionSpec
    from jax.experimental.shard_map import shard_map
    from concourse import bass2jax
    from concourse import mybir as mb

    bass2jax.install_neuronx_cc_hook()
    pn = nc.partition_id_tensor.name if nc.partition_id_tensor else None
    in_names, out_names, out_avals, out_shapes = [], [], [], []
    for alloc in nc.m.functions[0].allocations:
        if not isinstance(alloc, mb.MemoryLocationSet):
            continue
        name = alloc.memorylocations[0].name
        if alloc.kind == "ExternalInput":
            if name != pn:
                in_names.append(name)
        elif alloc.kind == "ExternalOutput":
            shape = tuple(alloc.tensor_shape)
            dtype = mb.dt.np(alloc.dtype)
            out_names.append(name)
            out_avals.append(jax.core.ShapedArray(shape, dtype))
            out_shapes.append((shape, dtype))
    n_params = len(in_names)
    n_outs = len(out_names)
    all_in = list(in_names) + list(out_names) + ([pn] if pn else [])

    def _body(*args):
        ops = list(args)
        if pn:
            ops.append(bass2jax.partition_id_tensor())
        return tuple(bass2jax._bass_exec_p.bind(
            *ops, out_avals=tuple(out_avals), in_names=tuple(all_in),
            out_names=tuple(out_names), lowering_input_output_aliases=(),
            sim_require_finite=True, sim_require_nnan=True, nc=nc))

    devices = jax.devices()[:n_cores]
    mesh = Mesh(np.asarray(devices), ("core",))
    specs = (PartitionSpec("core"),)
    fn = jax.jit(
        shard_map(_body, mesh=mesh, in_specs=specs * (n_params + n_outs),
                  out_specs=specs * n_outs, check_rep=False),
        donate_argnums=tuple(range(n_params, n_params + n_outs)) if donate else (),
        keep_unused=True)

    def run(in_maps):
        per_core = [[np.asarray(m[name]) for name in in_names] for m in in_maps]
        concat_in = [np.concatenate([per_core[c][i] for c in range(n_cores)],
                                    axis=0) for i in range(n_params)]
        concat_zeros = [np.zeros((n_cores * s[0], *s[1:]), d)
                        for (s, d) in out_shapes]
        import jax as _jax
        out_arrs = _jax.block_until_ready(fn(*concat_in, *concat_zeros))
        return [
            {name: np.asarray(out_arrs[i]).reshape(n_cores, *out_shapes[i][0])[c]
             for i, name in enumerate(out_names)}
            for c in range(n_cores)
        ]

    run.jit_fn = fn
    run.in_names = in_names
    run.out_names = out_names
    run.out_shapes = out_shapes
    run.n_cores = n_cores
    return run


def _unit_groups():
    units = [(b, r) for b in range(B) for r in range(A)]
    return [units[g * UNITS:(g + 1) * UNITS] for g in range(4)]


def shard_inputs(x, w_qkv, b_qkv, w_proj, b_proj):
    groups = _unit_groups()
    w4 = w_qkv.reshape(DIM, H, 3, D)
    b4 = b_qkv.reshape(H, 3, D)
    in_maps = []
    for c in range(NCORES):
        g, hh = c // 2, c % 2
        heads = list(range(hh * HL, (hh + 1) * HL))
        xT = np.ascontiguousarray(
            np.stack([x[b, :, r, :].T for (b, r) in groups[g]])
        ).astype(np.float32)
        wq = w4[:, heads, 0, :].reshape(DIM, HL * D) * SCALE
        wk = w4[:, heads, 1, :].reshape(DIM, HL * D) * SCALE
        wv = w4[:, heads, 2, :].reshape(DIM, HL * D)
        wqkv_c = np.ascontiguousarray(
            np.concatenate([wq, wk, wv], axis=1)).astype(np.float32)
        bq = (b4[heads, 0, :].reshape(HL * D) * SCALE)
        bk = (b4[heads, 1, :].reshape(HL * D) * SCALE)
        bvv = np.concatenate([b4[heads, 2, :], np.ones((HL, 1), np.float32)],
                             axis=1).reshape(HL * VW)
        in_maps.append({
            "xT": xT,
            "wqkv": wqkv_c,
            "bqk": np.concatenate([bq, bk]).astype(np.float32),
            "bv": bvv.astype(np.float32),
            "wproj": np.ascontiguousarray(
                w_proj[hh * HL * D:(hh + 1) * HL * D, :]).astype(
                    __import__("ml_dtypes").bfloat16),
        })
    return in_maps


def unshard(results, b_proj):
    groups = _unit_groups()
    out = np.zeros((B, N, A, DIM), np.float32)
    for g in range(4):
        s = (results[2 * g]["part"].astype(np.float32)
             + results[2 * g + 1]["part"].astype(np.float32))
        for idx, (b, r) in enumerate(groups[g]):
            out[b, :, r, :] = s[idx]
    return out + b_proj.astype(np.float32)


def get_runner(qk_bias=False, v_bias=False):
    key = ("runner", qk_bias, v_bias)
    if key not in _CACHE:
        nc = _build_nc(qk_bias=qk_bias, v_bias=v_bias)
        _CACHE[key] = _make_runner(nc)
    return _CACHE[key]


def kernel(x, w_qkv, b_qkv, w_proj, b_proj):
    x = np.asarray(x)
    w_qkv = np.asarray(w_qkv)
    b_qkv = np.asarray(b_qkv)
    w_proj = np.asarray(w_proj)
    b_proj = np.asarray(b_proj)
    run = get_runner(qk_bias=bool(np.any(b_qkv[:2048])),
                     v_bias=bool(np.any(b_qkv[2048:])))
    in_maps = shard_inputs(x, w_qkv, b_qkv, w_proj, b_proj)
    results = run(in_maps)
    return unshard(results, b_proj)

